# revision 54
# baseline (speedup 1.0000x reference)
"""Multi-head self-attention on 8 TRN2 NeuronCores.

Strategy: tensor-parallel over heads. Each core owns 2 of the 16 heads
(a 128-column slice of Wq/Wk/Wv and the matching 128-row slice of Wo) and
computes, for both batch elements:
  - its QKV projection columns, laid out transposed [cols, tokens],
  - full attention for its (batch, head) pairs via transposed scores
    [k, q] so every matmul keeps a 512-wide moving free dim,
  - a partial output projection against its Wo row-slice.
The host sums the 8 partial outputs (the tensor-parallel all-reduce) and
adds the output bias.

Performance structure (v2): the attention inner loop is kt-outer so each
stationary operand (K block, V' block) serves two 512-wide matmuls, scores
land in a 2-bank [128,1024] PSUM tile and are exponentiated by a single
1024-wide scalar-engine instruction (ACT is the attention-phase pacer).
PE filler work — batch-1 QKV chains during batch-0 attention, batch-0
output projection during batch-1 attention — keeps the tensor engine's
HAM activity monitor in the unthrottled 2.4 GHz state. V' tiles share one
ones-column between the two heads so the softmax normalizer falls out of
the attention matmul; the normalize multiply runs on the Pool engine off
the critical path.
"""

import numpy as np

B, S, D, H, DK = 2, 2048, 1024, 16, 64
NCORES = 8
HPC = H // NCORES          # heads per core = 2
COLS = HPC * DK            # feature columns per core = 128
T = B * S                  # 4096 tokens
TCH = 512                  # token chunk (max PSUM-bank moving free dim)
NT = T // TCH              # 8 token chunks
ND = D // 128              # 8 contraction chunks
NE = D // 128              # 8 output-column chunks
NQ = S // TCH              # 4 query chunks per batch
NQP = NQ // 2              # 2 query-pairs per batch
NKT = S // 128             # 16 key tiles per batch

_CACHE = {}


def _build_program():
    from contextlib import ExitStack

    import concourse.bacc as bacc
    import concourse.mybir as mybir
    import concourse.tile as tile
    from concourse.masks import make_identity

    f32 = mybir.dt.float32
    f32r = mybir.dt.float32r
    f16 = mybir.dt.float16
    Exp = mybir.ActivationFunctionType.Exp

    nc = bacc.Bacc("TRN2", target_bir_lowering=False, debug=False,
                   num_devices=NCORES)

    XT = nc.dram_tensor("xt", [D, T], f32, kind="ExternalInput").ap()
    Wq = nc.dram_tensor("wq", [D, COLS], f32, kind="ExternalInput").ap()
    Wk = nc.dram_tensor("wk", [D, COLS], f32, kind="ExternalInput").ap()
    Wv = nc.dram_tensor("wv", [D, COLS], f32, kind="ExternalInput").ap()
    Bq = nc.dram_tensor("bq", [COLS, 1], f32, kind="ExternalInput").ap()
    Bk = nc.dram_tensor("bk", [COLS, 1], f32, kind="ExternalInput").ap()
    Bv = nc.dram_tensor("bv", [COLS, 1], f32, kind="ExternalInput").ap()
    Wo = nc.dram_tensor("wo", [COLS, D], f32, kind="ExternalInput").ap()
    OT = nc.dram_tensor("ot", [D, T], f16, kind="ExternalOutput").ap()

    import concourse.bass as bass

    with tile.TileContext(nc) as tc, ExitStack() as ctx:
        consts = ctx.enter_context(tc.tile_pool(name="consts", bufs=1))
        xtp = ctx.enter_context(tc.tile_pool(name="xtp", bufs=24))
        big = ctx.enter_context(tc.tile_pool(name="big", bufs=1))
        expp = ctx.enter_context(tc.tile_pool(name="expp", bufs=3))
        attnp = ctx.enter_context(tc.tile_pool(name="attnp", bufs=8))
        arawp = ctx.enter_context(tc.tile_pool(name="arawp", bufs=4))
        zqp = ctx.enter_context(tc.tile_pool(name="zqp", bufs=4))
        rzp = ctx.enter_context(tc.tile_pool(name="rzp", bufs=2))
        obp = ctx.enter_context(tc.tile_pool(name="obp", bufs=8))
        ps_s = ctx.enter_context(tc.tile_pool(name="ps_s", bufs=2, space="PSUM"))
        ps_p = ctx.enter_context(tc.tile_pool(name="ps_p", bufs=2, space="PSUM"))
        ps_o = ctx.enter_context(tc.tile_pool(name="ps_o", bufs=2, space="PSUM"))

        # ---- constants; DMA issues (~620ns engine time apiece) are spread
        # over the otherwise-idle scalar/vector engines so the gpsimd/sync
        # X-tile streams start immediately ----
        w_sb = {}
        b_sb = {}

        def emit_w_dma(nm, src, bsrc, eng):
            w_sb[nm] = []
            for d in range(ND):
                wt = consts.tile([128, COLS], f32r, name=f"{nm}_{d}")
                eng.dma_start(wt, src[d * 128:(d + 1) * 128, :].bitcast(f32r))
                w_sb[nm].append(wt)
            bt = consts.tile([COLS, 1], f32, name=f"b{nm[1]}_sb")
            eng.dma_start(bt, bsrc)
            b_sb[nm] = bt

        emit_w_dma("wq", Wq, Bq, nc.scalar)
        emit_w_dma("wk", Wk, Bk, nc.scalar)
        ident = consts.tile([128, 128], f16, name="ident")
        make_identity(nc, ident)
        # broadcast mask for the softmax-normalizer matmul: spreads a
        # reciprocal row (partition 0 or 32) across 64 output partitions
        ones1 = consts.tile([33, DK], f16, name="ones1")
        nc.gpsimd.memset(ones1[0:1, :], 1.0)
        nc.gpsimd.memset(ones1[32:33, :], 1.0)

        # V' tiles: [keys 128, 2*DK+2] laid out [h0 dims | ones | h1 dims |
        # ones] so each head's 65-column lhsT slice puts the softmax
        # normalizer z at patt row DK.
        vpt = [[consts.tile([128, 2 * DK + 2], f16, name=f"vpt_{b}_{kt}")
                for kt in range(NKT)] for b in range(B)]

        QT = big.tile([COLS, T], f16, name="QT")
        KT = big.tile([COLS, T], f16, name="KT")
        VT = big.tile([COLS, T], f16, name="VT")
        proj_out = {"wq": QT, "wk": KT, "wv": VT}

        xts = {}

        def emit_xt_dma(t):
            tiles = []
            for d in range(ND):
                xt_t = xtp.tile([128, TCH], f32r, name=f"xt_{t}_{d}", tag="xt")
                eng = nc.gpsimd if d < 4 else nc.sync
                eng.dma_start(
                    xt_t,
                    XT[d * 128:(d + 1) * 128,
                       t * TCH:(t + 1) * TCH].bitcast(f32r))
                tiles.append(xt_t)
            xts[t] = tiles

        def emit_proj(t, nm):
            pacc = ps_o.tile([COLS, TCH], f32, tag="o", name=f"pacc_{t}_{nm}")
            for d in range(ND):
                nc.tensor.matmul(pacc, lhsT=w_sb[nm][d], rhs=xts[t][d],
                                 start=(d == 0), stop=(d == ND - 1))
            nc.vector.tensor_scalar_add(
                proj_out[nm][:, t * TCH:(t + 1) * TCH], pacc, b_sb[nm])

        def emit_vprep(b, kt):
            ptr = ps_o.tile([128, 128], f16, tag="o", name=f"ptr_{b}_{kt}")
            ks = slice(b * S + kt * 128, b * S + (kt + 1) * 128)
            nc.tensor.transpose(ptr, VT[:, ks], ident)
            v = vpt[b][kt]
            nc.vector.memset(v[:, DK:DK + 1], 1.0)
            nc.vector.memset(v[:, 2 * DK + 1:2 * DK + 2], 1.0)
            nc.vector.tensor_copy(v[:, 0:DK], ptr[:, 0:DK])
            nc.vector.tensor_copy(v[:, DK + 1:2 * DK + 1], ptr[:, DK:2 * DK])

        attn_sb = {}

        def emit_outproj(b, q, e):
            po = ps_o.tile([128, TCH], f32, tag="o", name=f"po_{b}_{q}_{e}")
            nc.tensor.matmul(po, lhsT=wo_sb[e], rhs=attn_sb[b, q],
                             start=True, stop=True)
            ob = obp.tile([128, TCH], f16, tag="osb", name=f"ob_{b}_{q}_{e}")
            nc.vector.tensor_copy(ob, po)
            ts_g = slice(b * S + q * TCH, b * S + (q + 1) * TCH)
            eng = nc.sync if e < 4 else nc.gpsimd
            eng.dma_start(OT[e * 128:(e + 1) * 128, ts_g], ob)

        def emit_attention(b, fillers=None, per_iter=2, late_fillers=None,
                           heads=tuple(range(HPC)), sched=None, rest=None,
                           rest_from=0):
            """Attention for batch b. Pops one filler every `per_iter`
            kt-iterations to keep the PE saturated past the ACT pacer.
            `late_fillers` are popped 2-per-iteration over the back half of
            the final kt loop (used for outproj units of this same batch,
            which depend on this batch's normalize chain)."""
            it = 0
            pending = []

            def emit_norm(h, q, ar, rz, row):
                # PE-broadcast of 1/z and the normalize multiply; deferred
                # into the next kt loop so the PE never waits on the DVE
                # drain/reciprocal chain
                zb = ps_o.tile([DK, TCH], f32, tag="o",
                               name=f"zb_{b}_{h}_{q}")
                nc.tensor.matmul(zb, lhsT=ones1[row:row + 1, :],
                                 rhs=rz[row:row + 1, :],
                                 start=True, stop=True)
                if (b, q) not in attn_sb:
                    attn_sb[b, q] = attnp.tile(
                        [COLS, TCH], f32r, tag="attn", name=f"attn_{b}_{q}")
                nc.vector.tensor_mul(attn_sb[b, q][h * DK:(h + 1) * DK, :],
                                     ar, zb[0:DK, :])

            for h in heads:
                hs = slice(h * DK, (h + 1) * DK)
                for qp in range(NQP):
                    last_loop = (h == heads[-1] and qp == NQP - 1)
                    patt = [ps_p.tile([DK + 1, TCH], f32, tag="patt",
                                      name=f"patt_{b}_{h}_{qp}_{j}")
                            for j in range(2)]
                    for kt in range(NKT):
                        ks = slice(b * S + kt * 128, b * S + (kt + 1) * 128)
                        sps = ps_s.tile([128, 2 * TCH], f32, tag="s",
                                        name=f"sc_{b}_{h}_{qp}_{kt}")
                        for j in range(2):
                            q = qp * 2 + j
                            qs = slice(b * S + q * TCH, b * S + (q + 1) * TCH)
                            nc.tensor.matmul(sps[:, j * TCH:(j + 1) * TCH],
                                             lhsT=KT[hs, ks], rhs=QT[hs, qs],
                                             start=True, stop=True)
                        esb = expp.tile([128, 2 * TCH], f16, tag="exp",
                                        name=f"exp_{b}_{h}_{qp}_{kt}")
                        nc.scalar.activation(esb, sps, Exp, scale=0.125)
                        vl = (vpt[b][kt][:, 0:DK + 1] if h == 0
                              else vpt[b][kt][:, DK + 1:2 * DK + 2])
                        for j in range(2):
                            nc.tensor.matmul(patt[j],
                                             lhsT=vl,
                                             rhs=esb[:, j * TCH:(j + 1) * TCH],
                                             start=(kt == 0),
                                             stop=(kt == NKT - 1))
                        it += 1
                        if kt == 4:
                            while pending:
                                pending.pop(0)()
                        if sched:
                            for f in sched.pop(it - 1, []):
                                f()
                        if rest and it - 1 >= rest_from:
                            rest.pop(0)()
                        if fillers and it % per_iter == 0:
                            fillers.pop(0)()
                        if (late_fillers and last_loop and kt >= 8
                                and len(late_fillers) > 10):
                            late_fillers.pop(0)()
                            if len(late_fillers) > 10:
                                late_fillers.pop(0)()
                    # drain: unnormalized attn rows, both normalizer rows
                    # gathered to partitions 0/32, one fast approximate
                    # reciprocal block, one cast — all off the PE stream
                    araw2 = []
                    zq2 = zqp.tile([DK, TCH], f32, tag="zq",
                                   name=f"zq_{b}_{h}_{qp}")
                    for j in range(2):
                        q = qp * 2 + j
                        ar = arawp.tile([DK, TCH], f32, tag="araw",
                                        name=f"araw_{b}_{h}_{q}")
                        nc.vector.tensor_copy(ar, patt[j][0:DK, :])
                        nc.vector.tensor_copy(zq2[32 * j:32 * j + 1, :],
                                              patt[j][DK:DK + 1, :])
                        araw2.append(ar)
                    zr32 = zqp.tile([DK, TCH], f32, tag="zq",
                                    name=f"zr32_{b}_{h}_{qp}")
                    nc.vector.reciprocal_approx_fast(zr32, zq2)
                    rz = rzp.tile([DK, TCH], f16, tag="rz",
                                  name=f"rz_{b}_{h}_{qp}")
                    nc.vector.tensor_copy(rz, zr32)
                    for j in range(2):
                        pending.append(
                            lambda h=h, q=qp * 2 + j, ar=araw2[j], rz=rz,
                            row=32 * j: emit_norm(h, q, ar, rz, row))
            # held-back late fillers bridge the final drain chain so the PE
            # has dependency-free work while the last normalize completes
            while late_fillers:
                late_fillers.pop(0)()
            while pending:
                pending.pop(0)()
            # leftover fillers
            while fillers:
                fillers.pop(0)()

        # ---- prologue: just enough QKV for batch-0 h0/qp0 to start ----
        emit_xt_dma(0)
        emit_xt_dma(1)
        emit_w_dma("wv", Wv, Bv, nc.gpsimd)
        wo_sb = []
        for e in range(NE):
            wt = consts.tile([128, 128], f32r, name=f"wo_{e}")
            nc.sync.dma_start(wt, Wo[:, e * 128:(e + 1) * 128].bitcast(f32r))
            wo_sb.append(wt)
        emit_xt_dma(2)
        emit_xt_dma(3)
        # proj(1,wq) first: it blocks on the t1 DMA (~11us), after which t0 is
        # fully resident and the rest of the prologue streams gap-free —
        # giving HAM its full-busy window early so everything runs at 2.4GHz
        emit_proj(1, "wq")
        for nm in ("wq", "wk", "wv"):
            emit_proj(0, nm)
        for kt in range(4):
            emit_vprep(0, kt)
        emit_proj(1, "wk")

        # ---- batch-0 attention; the rest of phase 1 rides inside it as
        # dependency-ordered fillers (kt needs key chunk kt//4 and vpt(kt)
        # a couple of iterations ahead) ----
        sched0 = {
            0: [lambda: emit_proj(1, "wv"),
                lambda: emit_vprep(0, 4), lambda: emit_vprep(0, 5)],
            1: [lambda: emit_vprep(0, 6), lambda: emit_vprep(0, 7)],
            2: [lambda: emit_proj(2, "wq")],
            3: [lambda: emit_proj(2, "wk")],
            4: [lambda: emit_proj(2, "wv")],
            5: [lambda: emit_vprep(0, 8), lambda: emit_vprep(0, 9)],
            6: [lambda: emit_vprep(0, 10), lambda: emit_vprep(0, 11)],
            7: [lambda: emit_proj(3, "wq")],
            8: [lambda: emit_proj(3, "wk")],
            9: [lambda: emit_proj(3, "wv")],
            10: [lambda: emit_vprep(0, 12), lambda: emit_vprep(0, 13)],
            11: [lambda: emit_vprep(0, 14), lambda: emit_vprep(0, 15)],
            13: [lambda: emit_xt_dma(4)],
            15: [lambda: emit_xt_dma(5)],
        }
        rest0 = []
        for t in range(4, NT):
            for nm in ("wq", "wk", "wv"):
                rest0.append(lambda t=t, nm=nm: emit_proj(t, nm))
            if t + 2 < NT:
                rest0.append(lambda t=t: emit_xt_dma(t + 2))
            b1kt = 4 * (t - 4)
            for kt in range(b1kt, b1kt + 4):
                rest0.append(lambda kt=kt: emit_vprep(1, kt))
        emit_attention(0, sched=sched0, rest=rest0, rest_from=16)

        # ---- batch-1 attention (heads in order h1,h0 so the late fillers
        # depend only on long-finished normalizes) with batch-0 output
        # projection as filler; batch-1's own q0/q1 outproj rides the tail
        # of the last kt loop ----
        fillers = []
        for q in range(NQ):
            for e in range(NE):
                fillers.append(lambda q=q, e=e: emit_outproj(0, q, e))
        # bridge the b0->b1 pipeline refill with dependency-free PE work
        for _ in range(4):
            fillers.pop(0)()
        late = []
        for q in range(2):
            for e in range(NE):
                late.append(lambda q=q, e=e: emit_outproj(1, q, e))
        emit_attention(1, fillers, per_iter=2, late_fillers=late,
                       heads=(1, 0))

        # ---- epilogue: batch-1 output projection, remaining quarters ----
        for q in range(2, NQ):
            for e in range(NE):
                emit_outproj(1, q, e)

    nc.compile()
    return nc


def _get_program():
    if "nc" not in _CACHE:
        _CACHE["nc"] = _build_program()
    return _CACHE["nc"]


def _install_ntff_hook():
    """Provide the antenv.axon_hooks shim this container's antenv lacks so
    run_bass_kernel_spmd(trace=True) can capture NTFF profiles."""
    import sys
    import types

    try:
        import antenv

        if hasattr(antenv, "axon_hooks"):
            return
        mod = types.ModuleType("antenv.axon_hooks")
        mod._hook = None
        mod.set_axon_ntff_profile_hook = lambda h: setattr(mod, "_hook", h)
        mod.get_axon_ntff_profile_hook = lambda: mod._hook
        sys.modules["antenv.axon_hooks"] = mod
        antenv.axon_hooks = mod
        from trn_agent_boot.trn_boot import _ntff_profile_via_ctypes

        mod.set_axon_ntff_profile_hook(
            _ntff_profile_via_ctypes("/opt/axon/libaxon_pjrt.so"))
    except Exception:
        pass


def kernel(X, Wq, bq, Wk, bk, Wv, bv, Wo, bo, _profile=False, _trace_cores=None):
    from concourse.bass_utils import run_bass_kernel_spmd

    if _profile:
        _install_ntff_hook()

    nc = _get_program()

    XT = np.ascontiguousarray(np.asarray(X, np.float32).reshape(T, D).T)
    Wq, Wk, Wv, Wo = (np.asarray(w, np.float32) for w in (Wq, Wk, Wv, Wo))
    bq, bk, bv, bo = (np.asarray(v, np.float32) for v in (bq, bk, bv, bo))

    in_maps = []
    for c in range(NCORES):
        cs = slice(c * COLS, (c + 1) * COLS)
        in_maps.append({
            "xt": XT,
            "wq": np.ascontiguousarray(Wq[:, cs]),
            "wk": np.ascontiguousarray(Wk[:, cs]),
            "wv": np.ascontiguousarray(Wv[:, cs]),
            "bq": np.ascontiguousarray(bq[cs].reshape(COLS, 1)),
            "bk": np.ascontiguousarray(bk[cs].reshape(COLS, 1)),
            "bv": np.ascontiguousarray(bv[cs].reshape(COLS, 1)),
            "wo": np.ascontiguousarray(Wo[cs, :]),
        })

    res = run_bass_kernel_spmd(
        nc, in_maps, core_ids=list(range(NCORES)),
        trace=_profile,
        trace_cores=(_trace_cores if _trace_cores is not None
                     else ([0] if _profile else None)),
    )

    ot = res.results[0]["ot"].astype(np.float64)
    for c in range(1, NCORES):
        ot += res.results[c]["ot"].astype(np.float64)
    out = (ot.T + bo).astype(np.float32).reshape(B, S, D)
    if _profile:
        kernel.last_exec_time_ns = res.exec_time_ns
        kernel.last_results = res
    return out
